# revision 40
# baseline (speedup 1.0000x reference)
"""ClusterNorm1dv2 training-mode forward on 8 trn2 NeuronCores.

Sharding: over clusters K (16 clusters per core, full batch) -- no
collectives at all.  The host hands each core a contiguous bf16 slab
xs[b, k'*32+d] (cluster-major columns).  Pass 1 streams the slab into a
resident SBUF bf16 buffer while accumulating per-cluster second moments
(4 group matmuls per 128-row tile: the 32x32 diagonal sub-blocks of
each [128,128] group product are the S_k) and column sums (ones-vector
matmul) in PSUM.  The tiny [16,D,D] covariance assembly + LDL^T
factorization + unit-triangular inversion runs vectorized over the 16
clusters on partitions 0..15 (vector engine).  While that serial chain
runs, the PE transposes every resident tile group and the scalar engine
copies the transposes back IN PLACE over the resident buffer (x is dead
after stats+transpose), so pass 2 starts with all operands staged.
Pass 2 whitens with one [128x128]x[128,512] bf16 matmul per (chunk,
group) against a block-diagonal W, adds the -W@mu bias per partition
(alternating vector/scalar), and streams z^T out in bf16 (host upcasts)
via gpsimd/sync-alternating DMA triggers.  Host does all layout
shuffles / dtype casts (not part of the measured NEFF execution).
"""

import numpy as np
import ml_dtypes

import concourse.bacc as bacc
import concourse.mybir as mybir
import concourse.tile as tile
from concourse.bass_utils import run_bass_kernel_spmd

F32 = mybir.dt.float32
BF16 = mybir.dt.bfloat16
ALU = mybir.AluOpType
ACTF = mybir.ActivationFunctionType

N_CORES = 8
B, D, K = 16384, 32, 128
KC = K // N_CORES          # 16 clusters per core
COLS = KC * D              # 512 columns per core slab
NT = B // 128              # 128 tiles of [128, 512]
P = 128
NCH = 32                   # chunks (4 tiles = 512 batch rows each)
DD = D * D                 # 1024

_CACHE = {}


def _build():
    nc = bacc.Bacc("TRN2", target_bir_lowering=False, debug=False,
                   num_devices=N_CORES)

    xs = nc.dram_tensor("xs", [B, COLS], BF16, kind="ExternalInput")
    xs8 = nc.dram_tensor("xs8", [B, COLS], mybir.dt.float8e4,
                         kind="ExternalInput")
    ghat_in = nc.dram_tensor("ghat_in", [KC, DD], F32, kind="ExternalInput")
    n0mu0_in = nc.dram_tensor("n0mu0_in", [KC, D], F32, kind="ExternalInput")
    mu0t_in = nc.dram_tensor("mu0t_in", [KC, D], F32, kind="ExternalInput")
    scal_in = nc.dram_tensor("scal_in", [1, 2], F32, kind="ExternalInput")
    eye_in = nc.dram_tensor("eye_in", [KC, DD], F32, kind="ExternalInput")
    idt_in = nc.dram_tensor("idt_in", [P, P], BF16, kind="ExternalInput")
    ones_in = nc.dram_tensor("ones_in", [P, 1], BF16, kind="ExternalInput")
    ones8_in = nc.dram_tensor("ones8_in", [P, 128], mybir.dt.float8e4,
                              kind="ExternalInput")
    zt_out = nc.dram_tensor("zt_out", [COLS, B], BF16, kind="ExternalOutput")

    with tile.TileContext(nc) as tc:
        with (
            tc.tile_pool(name="consts", bufs=1) as consts,
            tc.tile_pool(name="resid", bufs=1) as resid,
            tc.tile_pool(name="chain", bufs=1) as chp,
            tc.tile_pool(name="chtmp", bufs=2) as chtmp,
            tc.tile_pool(name="zb0", bufs=2) as zb0,
            tc.tile_pool(name="zb1", bufs=2) as zb1,
            tc.tile_pool(name="zb2", bufs=2) as zb2,
            tc.tile_pool(name="zb3", bufs=2) as zb3,
            tc.tile_pool(name="dram", bufs=1, space="DRAM") as dr,
        ):
            # constants needed early (PE transpose identity, fp8 sums ones)
            # -- on the scalar queue so the fp8 x stream starts immediately
            idt = consts.tile([P, P], BF16, tag="idt")
            nc.scalar.dma_start(idt[:], idt_in[:])
            ob8 = consts.tile([P, 128], mybir.dt.float8e4, tag="ob8")
            nc.scalar.dma_start(ob8[:], ones8_in[:])

            # ---------------- pass 1: fp8 stats (DoubleRow matmuls) -------
            # Stream a streamed fp8 copy of x; each DoubleRow matmul
            # contracts 256 batch rows (two tiles) at 0.5 cycles/row.
            xbt = [resid.tile([P, 4 * COLS], BF16, tag=f"xb{ci}",
                              name=f"xb{ci}") for ci in range(NCH)]

            def xbv(t):
                # [128, 512] bf16 view of b-tile t
                return xbt[t // 4][:, COLS * (t % 4): COLS * (t % 4 + 1)]

            with (
                tc.tile_pool(name="x8p", bufs=3) as x8p,
                tc.tile_pool(name="prodp", bufs=1, space="PSUM") as prodp,
                tc.tile_pool(name="sumsp", bufs=1, space="PSUM") as sumsp,
            ):
                prod = prodp.tile([P, COLS], F32, tag="prod")
                sums = sumsp.tile([64, COLS], F32, tag="sums")
                for ci in range(NCH):
                    x8 = x8p.tile([P, 4 * COLS], mybir.dt.float8e4,
                                  tag="x8")
                    nc.sync.dma_start(
                        x8[:].rearrange("p (j c) -> p j c", j=4),
                        xs8[:][512 * ci: 512 * (ci + 1), :].rearrange(
                            "(j p) c -> p j c", j=4),
                    )
                    x8v = x8[:].rearrange("p (q two c) -> p q two c",
                                          q=2, two=2)
                    sp = ci == NCH - 1
                    for q in range(2):
                        for g in range(4):
                            sel = x8v[:, q, :, 128 * g: 128 * (g + 1)]
                            nc.tensor.matmul(
                                prod[:, 128 * g: 128 * (g + 1)],
                                sel, sel,
                                start=(ci == 0 and q == 0 and g == 0),
                                stop=(sp and q == 1),
                                perf_mode=mybir.MatmulPerfMode.DoubleRow,
                                skip_group_check=True,
                            )
                        nc.tensor.matmul(
                            sums[:],
                            ob8[:].rearrange("p (two m) -> p two m", two=2),
                            x8v[:, q],
                            start=(ci == 0 and q == 0),
                            stop=(sp and q == 1),
                            perf_mode=mybir.MatmulPerfMode.DoubleRow,
                            skip_group_check=True,
                        )

                # extract stats to SBUF
                s_sb = consts.tile([P, COLS], F32, tag="s_sb")
                nc.vector.tensor_copy(s_sb[:], prod[:])
                t_sb = consts.tile([1, COLS], F32, tag="t_sb")
                nc.scalar.copy(t_sb[:], sums[0:1, :])

            # remaining constants
            wblk = consts.tile([P, COLS], BF16, tag="wblk")
            nc.gpsimd.memset(wblk[:], 0.0)
            bias = consts.tile([P, 4], F32, tag="bias")
            invden = consts.tile([KC, 1], F32, tag="invden")
            nc.gpsimd.dma_start(
                invden[:], scal_in[:][0:1, 0:1].broadcast_to([KC, 1]))
            coefx = consts.tile([KC, 1], F32, tag="coefx")
            nc.gpsimd.dma_start(
                coefx[:], scal_in[:][0:1, 1:2].broadcast_to([KC, 1]))
            ghat = chp.tile([KC, DD], F32, tag="ghat")
            nc.gpsimd.dma_start(ghat[:], ghat_in[:])
            n0mu0 = chp.tile([KC, D], F32, tag="n0mu0")
            nc.gpsimd.dma_start(n0mu0[:], n0mu0_in[:])
            mu0t = chp.tile([KC, D], F32, tag="mu0t")
            nc.gpsimd.dma_start(mu0t[:], mu0t_in[:])
            wu = chp.tile([KC, DD], F32, tag="wu")
            nc.gpsimd.dma_start(wu[:], eye_in[:])

            # scatter stats into cluster-per-partition chain layout via DRAM
            # (t path on the scalar engine's DMA queue, parallel to the
            # s path on sync)
            t_dr = dr.tile([KC, D], F32, tag="t_dr")
            nc.scalar.dma_start(
                t_dr[:].rearrange("k d -> (k d)").unsqueeze(0),
                t_sb[0:1, :])
            t_k = chp.tile([KC, D], F32, tag="t_k")
            nc.scalar.dma_start(t_k[:], t_dr[:])
            # s path on the vector engine's DMA queue: the sync queues are
            # still draining the bf16 x transfers at this point.
            s_dr = dr.tile([KC, DD], F32, tag="s_dr")
            for i in range(4):
                nc.gpsimd.dma_start(
                    s_dr[:].rearrange("(g f) c -> f g c", f=4)[i]
                    .rearrange("g (e d) -> e g d", d=D),
                    s_sb[32 * i: 32 * (i + 1), :].rearrange(
                        "e (g c) -> e g c", c=128)[:, :, 32 * i: 32 * i + 32],
                )
            am = chp.tile([KC, DD], F32, tag="am")
            nc.gpsimd.dma_start(am[:], s_dr[:])

            # bf16 x input DMAs: queued on sync after the stat scatters so
            # the transfers run during the factorization window (DMA is
            # idle there); pass-2 transposes pick each chunk up as it lands.
            for ci in range(NCH):
                nc.sync.dma_start(
                    xbt[ci][:].rearrange("p (j c) -> p j c", j=4),
                    xs[:][512 * ci: 512 * (ci + 1), :].rearrange(
                        "(j p) c -> p j c", j=4),
                )

            # ---------------- cov assembly (am = new_cov + I) ----------------
            av = am[:].rearrange("p (e d) -> p e d", d=D)
            xbar = chp.tile([KC, D], F32, tag="xbar")
            nc.vector.tensor_scalar_mul(xbar[:], t_k[:], 1.0 / B)
            xd = chp.tile([KC, D], F32, tag="xd")
            nc.vector.tensor_sub(xd[:], xbar[:], mu0t[:])
            nmu = chp.tile([KC, D], F32, tag="nmu")
            nc.vector.tensor_add(nmu[:], n0mu0[:], t_k[:])
            nc.vector.tensor_scalar_mul(nmu[:], nmu[:], invden[:])
            tmp1 = chp.tile([KC, DD], F32, tag="tmp1")
            tv = tmp1[:].rearrange("p (e d) -> p e d", d=D)
            nc.vector.tensor_tensor(
                tv,
                t_k[:].unsqueeze(2).broadcast_to([KC, D, D]),
                xbar[:].unsqueeze(1).broadcast_to([KC, D, D]),
                ALU.mult,
            )
            nc.vector.tensor_sub(am[:], am[:], tmp1[:])
            nc.vector.scalar_tensor_tensor(
                am[:], am[:], invden[:], ghat[:], ALU.mult, ALU.add)
            nc.vector.tensor_tensor(
                tv,
                xd[:].unsqueeze(2).broadcast_to([KC, D, D]),
                xd[:].unsqueeze(1).broadcast_to([KC, D, D]),
                ALU.mult,
            )
            nc.vector.scalar_tensor_tensor(
                am[:], tmp1[:], coefx[:], am[:], ALU.mult, ALU.add)

            # ---------------- LDL^T factorization (vector engine) ----------
            for j in range(D - 1):
                n = D - 1 - j
                rawc = am[:, 32 * (j + 1) + j: DD: 32]
                invd = chtmp.tile([KC, 1], F32, tag="invd")
                nc.vector.reciprocal(invd[:], am[:, 33 * j: 33 * j + 1])
                nc.vector.tensor_scalar_mul(invd[:], invd[:], -1.0)
                tmpu = chtmp.tile([KC, 31, 31], F32, tag="tmpu")
                nc.vector.tensor_tensor(
                    tmpu[:, 0:n, 0:n],
                    rawc.unsqueeze(2).broadcast_to([KC, n, n]),
                    rawc.unsqueeze(1).broadcast_to([KC, n, n]),
                    ALU.mult,
                )
                nc.vector.scalar_tensor_tensor(
                    av[:, j + 1: D, j + 1: D],
                    tmpu[:, 0:n, 0:n],
                    invd[:],
                    av[:, j + 1: D, j + 1: D],
                    ALU.mult, ALU.add,
                )

            dv = chp.tile([KC, D], F32, tag="dv")
            nc.vector.tensor_copy(dv[:], am[:, 0:DD:33])
            rdv = chp.tile([KC, D], F32, tag="rdv")
            nc.vector.reciprocal(rdv[:], dv[:])
            # unit-lower L: scale columns by 1/d (upper/diag junk unused)
            ltmp = tmp1
            nc.vector.tensor_tensor(
                ltmp[:].rearrange("p (e d) -> p e d", d=D),
                av,
                rdv[:].unsqueeze(1).broadcast_to([KC, D, D]),
                ALU.mult,
            )

            # ------------- pass-2 prep: transpose resident x IN PLACE -------
            # Emitted here so PE/scalar overlap the vector-engine chain.
            # (The scalar sqrt below is intentionally AFTER most copies in
            # the scalar queue: rsq isn't needed until the final row scale.)
            NCH_EARLY = 18   # rest emitted after the inverse loop (fills the
            #                  PE idle gap so it stays warm for the whitens)
            with tc.tile_pool(name="xps", bufs=2, space="PSUM") as xps:

                def emit_xpose(c):
                    pxt = xps.tile([P, 4 * COLS], BF16, tag="pxt")
                    for j in range(4):
                        xt = xbv(4 * c + j)
                        for g in range(4):
                            nc.tensor.transpose(
                                pxt[:, 512 * g + 128 * j:
                                    512 * g + 128 * (j + 1)],
                                xt[:, 128 * g: 128 * (g + 1)],
                                idt[:],
                            )
                    nc.scalar.copy(xbt[c][:], pxt[:])

                for c in range(NCH_EARLY):
                    emit_xpose(c)

                # rsq = 1/sqrt(d): scalar sqrt + 2 Newton steps on vector
                rsq = chp.tile([KC, D], F32, tag="rsq")
                nc.scalar.activation(rsq[:], rdv[:], ACTF.Sqrt)

                # ---------------- unit-lower inverse ----------------
                wv = wu[:].rearrange("p (i c) -> p i c", c=D)
                for jc in range(D - 1):
                    n = D - 1 - jc
                    lcol = ltmp[:, 32 * (jc + 1) + jc: DD: 32]
                    roww = wv[:, jc, 0: jc + 1]
                    tmpu = chtmp.tile([KC, 31, 31], F32, tag="tmpu")
                    nc.vector.tensor_tensor(
                        tmpu[:, 0:n, 0: jc + 1],
                        lcol.unsqueeze(2).broadcast_to([KC, n, jc + 1]),
                        roww.unsqueeze(1).broadcast_to([KC, n, jc + 1]),
                        ALU.mult,
                    )
                    nc.vector.tensor_sub(
                        wv[:, jc + 1: D, 0: jc + 1],
                        wv[:, jc + 1: D, 0: jc + 1],
                        tmpu[:, 0:n, 0: jc + 1],
                    )

                for c in range(NCH_EARLY, NCH):
                    emit_xpose(c)

                nt1 = chp.tile([KC, D], F32, tag="nt1")
                for _ in range(2):
                    nc.vector.tensor_tensor(nt1[:], rsq[:], rsq[:], ALU.mult)
                    nc.vector.tensor_tensor(nt1[:], nt1[:], dv[:], ALU.mult)
                    nc.vector.tensor_scalar(
                        out=nt1[:], in0=nt1[:], scalar1=-0.5, scalar2=1.5,
                        op0=ALU.mult, op1=ALU.add,
                    )
                    nc.vector.tensor_tensor(rsq[:], rsq[:], nt1[:], ALU.mult)

                # scale rows by 1/sqrt(d)
                nc.vector.tensor_tensor(
                    wv, wv,
                    rsq[:].unsqueeze(2).broadcast_to([KC, D, D]), ALU.mult)

                # W^T (e-major) in bf16, scattered to block-diag wblk first
                # (the whitens need wblk; the bias path can lag)
                wt16 = chp.tile([KC, DD], BF16, tag="wt16")
                nc.vector.tensor_copy(
                    wt16[:].rearrange("p (e d) -> p e d", d=D),
                    wv.transpose([0, 2, 1]),
                )
                wt_dr = dr.tile([KC, DD], BF16, tag="wt_dr")
                nc.sync.dma_start(wt_dr[:], wt16[:])
                for i in range(4):
                    nc.sync.dma_start(
                        wblk[32 * i: 32 * (i + 1), :].rearrange(
                            "e (g c) -> e g c", c=128)[
                                :, :, 32 * i: 32 * i + 32],
                        wt_dr[:].rearrange("(g f) c -> f g c", f=4)[i]
                        .rearrange("g (e d) -> e g d", d=D),
                    )

                # bias = -W @ new_mu  (per cluster)
                nc.vector.tensor_tensor(
                    ltmp[:].rearrange("p (d e) -> p d e", e=D),
                    wv,
                    nmu[:].unsqueeze(1).broadcast_to([KC, D, D]),
                    ALU.mult,
                )
                wmu = chp.tile([KC, D], F32, tag="wmu")
                nc.vector.tensor_reduce(
                    wmu[:], ltmp[:].rearrange("p (d e) -> p d e", e=D),
                    mybir.AxisListType.X, ALU.add,
                )
                nc.vector.tensor_scalar_mul(wmu[:], wmu[:], -1.0)
                wm_dr = dr.tile([KC, D], F32, tag="wm_dr")
                nc.sync.dma_start(wm_dr[:], wmu[:])
                # flat(wm_dr)[k'*32+d] = flat[128*g + (32*i+d)] -> [p, g]
                nc.sync.dma_start(
                    bias[:],
                    wm_dr[:].rearrange("(g i) d -> g (i d)", i=4)
                    .transpose([1, 0]),
                )

            # ---------------- pass 2: whiten ----------------
            with (
                tc.tile_pool(name="zps", bufs=7, space="PSUM") as zps,
                tc.tile_pool(name="wrm", bufs=1, space="PSUM") as wrm,
            ):
                # PE p-state warm-up: ~3us of dummy transposes gated on
                # wblk so the whitens start at full clock.
                scr = wrm.tile([1, P], BF16, tag="scr")
                for _ in range(20):
                    nc.tensor.transpose(scr[:], wblk[:, 0:1], idt[:])
                # z staging: per-group [128, 1024] tiles covering chunk
                # pairs -> 64 output DMAs, all on the (otherwise idle) sync
                # queue.  Bias adds rotate scalar/vector/gpsimd.
                zstp = [zb0, zb1, zb2, zb3]
                tg = [None] * 4
                for c in range(NCH):
                    for g in range(4):
                        pz = zps.tile([P, 512], F32, tag="pz")
                        nc.tensor.matmul(
                            pz[:],
                            wblk[:, 128 * g: 128 * (g + 1)],
                            xbt[c][:, 512 * g: 512 * (g + 1)],
                            start=True, stop=True,
                        )
                        if c % 2 == 0:
                            tg[g] = zstp[g].tile([P, 1024], BF16,
                                                 tag=f"zb{g}",
                                                 name=f"zb{g}_{c}")
                        half = tg[g][:, 512 * (c % 2): 512 * (c % 2 + 1)]
                        on_scalar = (g == 0) or (g == 2 and c % 2 == 0) \
                            or (g == 3 and c % 2 == 1)
                        if on_scalar:
                            nc.scalar.activation(
                                half, pz[:], ACTF.Identity,
                                bias=bias[:, g: g + 1])
                        else:
                            nc.vector.tensor_scalar_add(
                                half, pz[:], bias[:, g: g + 1])
                        if c % 2 == 1:
                            nc.sync.dma_start(
                                zt_out[:][128 * g: 128 * (g + 1),
                                          512 * (c - 1): 512 * (c + 1)],
                                tg[g][:],
                            )

    nc.compile()
    return nc


def _get_nc():
    if "nc" not in _CACHE:
        _CACHE["nc"] = _build()
    return _CACHE["nc"]


def kernel(x, mu_0, L_0, n_0):
    x = np.asarray(x, dtype=np.float32)
    mu_0 = np.asarray(mu_0, dtype=np.float32)
    L_0 = np.asarray(L_0, dtype=np.float32)
    n_0 = np.asarray(n_0, dtype=np.float32)

    nc = _get_nc()

    n0 = float(n_0[0])
    denom = n0 + B
    invden = 1.0 / denom
    coefg = n0 / denom
    coefx = n0 * B / (denom * denom)
    scal = np.array([[invden, coefx]], dtype=np.float32)
    idt = np.eye(P, dtype=ml_dtypes.bfloat16)
    ones = np.ones((P, 1), dtype=ml_dtypes.bfloat16)
    fp8 = mybir.dt.np(mybir.dt.float8e4)
    ones8 = np.ones((P, 128), dtype=fp8)
    eye = np.broadcast_to(
        np.eye(D, dtype=np.float32).reshape(1, DD), (KC, DD)).copy()
    mu0t_full = np.ascontiguousarray(mu_0.T)          # [K, D]
    g_full = np.einsum('kde,kfe->kdf', L_0, L_0)      # [K, D, D]

    # per-core slabs: xr2[c] = [B, 512] cluster-major (col = k'*32 + d)
    xr = np.ascontiguousarray(x.transpose(0, 2, 1))   # [B, K, D]
    xr2 = np.ascontiguousarray(
        xr.reshape(B, N_CORES, COLS).transpose(1, 0, 2))  # [8, B, 512]

    in_maps = []
    for c in range(N_CORES):
        sl = slice(KC * c, KC * (c + 1))
        ghat = (g_full[sl].reshape(KC, DD) * coefg
                + eye).astype(np.float32)
        in_maps.append({
            "xs": xr2[c].astype(ml_dtypes.bfloat16),
            "xs8": xr2[c].astype(fp8),
            "ones8_in": ones8,
            "ghat_in": np.ascontiguousarray(ghat),
            "n0mu0_in": np.ascontiguousarray(n0 * mu0t_full[sl]),
            "mu0t_in": np.ascontiguousarray(mu0t_full[sl]),
            "scal_in": scal,
            "eye_in": eye,
            "idt_in": idt,
            "ones_in": ones,
        })
    res = run_bass_kernel_spmd(
        nc, in_maps, core_ids=list(range(N_CORES)),
        trace=bool(_CACHE.get("trace", False)),
    )
    _CACHE["last_res"] = res

    z = np.empty((B, D, K), dtype=np.float32)
    for c in range(N_CORES):
        zt = np.asarray(res.results[c]["zt_out"],
                        dtype=np.float32)            # [512, B]
        # row = 128*g + 32*i + d  ->  cluster k' = 4*g + i, feature d
        zc = zt.reshape(4, 4, D, B).transpose(3, 2, 0, 1).reshape(B, D, KC)
        z[:, :, KC * c: KC * (c + 1)] = zc
    return z


# revision 41
# speedup vs baseline: 1.0702x; 1.0702x over previous
"""ClusterNorm1dv2 training-mode forward on 8 trn2 NeuronCores.

Sharding: over clusters K (16 clusters per core, full batch) -- no
collectives at all.  The host hands each core a contiguous bf16 slab
xs[b, k'*32+d] (cluster-major columns).  Pass 1 streams the slab into a
resident SBUF bf16 buffer while accumulating per-cluster second moments
(4 group matmuls per 128-row tile: the 32x32 diagonal sub-blocks of
each [128,128] group product are the S_k) and column sums (ones-vector
matmul) in PSUM.  The tiny [16,D,D] covariance assembly + LDL^T
factorization + unit-triangular inversion runs vectorized over the 16
clusters on partitions 0..15 (vector engine).  While that serial chain
runs, the PE transposes every resident tile group and the scalar engine
copies the transposes back IN PLACE over the resident buffer (x is dead
after stats+transpose), so pass 2 starts with all operands staged.
Pass 2 whitens with one [128x128]x[128,512] bf16 matmul per (chunk,
group) against a block-diagonal W, adds the -W@mu bias per partition
(alternating vector/scalar), and streams z^T out in bf16 (host upcasts)
via gpsimd/sync-alternating DMA triggers.  Host does all layout
shuffles / dtype casts (not part of the measured NEFF execution).
"""

import numpy as np
import ml_dtypes

import concourse.bacc as bacc
import concourse.mybir as mybir
import concourse.tile as tile
from concourse.bass_utils import run_bass_kernel_spmd

F32 = mybir.dt.float32
BF16 = mybir.dt.bfloat16
ALU = mybir.AluOpType
ACTF = mybir.ActivationFunctionType

N_CORES = 8
B, D, K = 16384, 32, 128
KC = K // N_CORES          # 16 clusters per core
COLS = KC * D              # 512 columns per core slab
NT = B // 128              # 128 tiles of [128, 512]
P = 128
NCH = 32                   # chunks (4 tiles = 512 batch rows each)
DD = D * D                 # 1024

_CACHE = {}


def _build():
    nc = bacc.Bacc("TRN2", target_bir_lowering=False, debug=False,
                   num_devices=N_CORES)

    xs = nc.dram_tensor("xs", [B, COLS], BF16, kind="ExternalInput")
    xs8 = nc.dram_tensor("xs8", [B, COLS], mybir.dt.float8e4,
                         kind="ExternalInput")
    ghat_in = nc.dram_tensor("ghat_in", [KC, DD], F32, kind="ExternalInput")
    n0mu0_in = nc.dram_tensor("n0mu0_in", [KC, D], F32, kind="ExternalInput")
    mu0t_in = nc.dram_tensor("mu0t_in", [KC, D], F32, kind="ExternalInput")
    scal_in = nc.dram_tensor("scal_in", [1, 2], F32, kind="ExternalInput")
    eye_in = nc.dram_tensor("eye_in", [KC, DD], F32, kind="ExternalInput")
    idt_in = nc.dram_tensor("idt_in", [P, P], BF16, kind="ExternalInput")
    ones_in = nc.dram_tensor("ones_in", [P, 1], BF16, kind="ExternalInput")
    ones8_in = nc.dram_tensor("ones8_in", [P, 128], mybir.dt.float8e4,
                              kind="ExternalInput")
    zt_out = nc.dram_tensor("zt_out", [COLS, B], BF16, kind="ExternalOutput")

    with tile.TileContext(nc) as tc:
        with (
            tc.tile_pool(name="consts", bufs=1) as consts,
            tc.tile_pool(name="resid", bufs=1) as resid,
            tc.tile_pool(name="chain", bufs=1) as chp,
            tc.tile_pool(name="chtmp", bufs=2) as chtmp,
            tc.tile_pool(name="zb0", bufs=2) as zb0,
            tc.tile_pool(name="zb1", bufs=2) as zb1,
            tc.tile_pool(name="zb2", bufs=2) as zb2,
            tc.tile_pool(name="zb3", bufs=2) as zb3,
            tc.tile_pool(name="dram", bufs=1, space="DRAM") as dr,
        ):
            # constants needed early (PE transpose identity, fp8 sums ones)
            idt = consts.tile([P, P], BF16, tag="idt")
            nc.sync.dma_start(idt[:], idt_in[:])
            ob8 = consts.tile([P, 128], mybir.dt.float8e4, tag="ob8")
            nc.sync.dma_start(ob8[:], ones8_in[:])

            # ---------------- pass 1: fp8 stats (DoubleRow matmuls) -------
            # Stream a streamed fp8 copy of x; each DoubleRow matmul
            # contracts 256 batch rows (two tiles) at 0.5 cycles/row.
            xbt = [resid.tile([P, 4 * COLS], BF16, tag=f"xb{ci}",
                              name=f"xb{ci}") for ci in range(NCH)]

            def xbv(t):
                # [128, 512] bf16 view of b-tile t
                return xbt[t // 4][:, COLS * (t % 4): COLS * (t % 4 + 1)]

            with (
                tc.tile_pool(name="x8p", bufs=3) as x8p,
                tc.tile_pool(name="prodp", bufs=1, space="PSUM") as prodp,
                tc.tile_pool(name="sumsp", bufs=1, space="PSUM") as sumsp,
            ):
                prod = prodp.tile([P, COLS], F32, tag="prod")
                sums = sumsp.tile([64, COLS], F32, tag="sums")
                for ci in range(NCH):
                    x8 = x8p.tile([P, 4 * COLS], mybir.dt.float8e4,
                                  tag="x8")
                    nc.sync.dma_start(
                        x8[:].rearrange("p (j c) -> p j c", j=4),
                        xs8[:][512 * ci: 512 * (ci + 1), :].rearrange(
                            "(j p) c -> p j c", j=4),
                    )
                    x8v = x8[:].rearrange("p (q two c) -> p q two c",
                                          q=2, two=2)
                    sp = ci == NCH - 1
                    for q in range(2):
                        for g in range(4):
                            sel = x8v[:, q, :, 128 * g: 128 * (g + 1)]
                            nc.tensor.matmul(
                                prod[:, 128 * g: 128 * (g + 1)],
                                sel, sel,
                                start=(ci == 0 and q == 0 and g == 0),
                                stop=(sp and q == 1),
                                perf_mode=mybir.MatmulPerfMode.DoubleRow,
                                skip_group_check=True,
                            )
                        nc.tensor.matmul(
                            sums[:],
                            ob8[:].rearrange("p (two m) -> p two m", two=2),
                            x8v[:, q],
                            start=(ci == 0 and q == 0),
                            stop=(sp and q == 1),
                            perf_mode=mybir.MatmulPerfMode.DoubleRow,
                            skip_group_check=True,
                        )

                # extract stats to SBUF
                s_sb = consts.tile([P, COLS], F32, tag="s_sb")
                nc.vector.tensor_copy(s_sb[:], prod[:])
                t_sb = consts.tile([1, COLS], F32, tag="t_sb")
                nc.scalar.copy(t_sb[:], sums[0:1, :])

            # remaining constants
            wblk = consts.tile([P, COLS], BF16, tag="wblk")
            nc.gpsimd.memset(wblk[:], 0.0)
            bias = consts.tile([P, 4], F32, tag="bias")
            invden = consts.tile([KC, 1], F32, tag="invden")
            nc.gpsimd.dma_start(
                invden[:], scal_in[:][0:1, 0:1].broadcast_to([KC, 1]))
            coefx = consts.tile([KC, 1], F32, tag="coefx")
            nc.gpsimd.dma_start(
                coefx[:], scal_in[:][0:1, 1:2].broadcast_to([KC, 1]))
            ghat = chp.tile([KC, DD], F32, tag="ghat")
            nc.gpsimd.dma_start(ghat[:], ghat_in[:])
            n0mu0 = chp.tile([KC, D], F32, tag="n0mu0")
            nc.gpsimd.dma_start(n0mu0[:], n0mu0_in[:])
            mu0t = chp.tile([KC, D], F32, tag="mu0t")
            nc.gpsimd.dma_start(mu0t[:], mu0t_in[:])
            wu = chp.tile([KC, DD], F32, tag="wu")
            nc.gpsimd.dma_start(wu[:], eye_in[:])

            # scatter stats into cluster-per-partition chain layout via DRAM
            # (t path on the scalar engine's DMA queue, parallel to the
            # s path on sync)
            t_dr = dr.tile([KC, D], F32, tag="t_dr")
            nc.scalar.dma_start(
                t_dr[:].rearrange("k d -> (k d)").unsqueeze(0),
                t_sb[0:1, :])
            t_k = chp.tile([KC, D], F32, tag="t_k")
            nc.scalar.dma_start(t_k[:], t_dr[:])
            s_dr = dr.tile([KC, DD], F32, tag="s_dr")
            for i in range(4):
                nc.sync.dma_start(
                    s_dr[:].rearrange("(g f) c -> f g c", f=4)[i]
                    .rearrange("g (e d) -> e g d", d=D),
                    s_sb[32 * i: 32 * (i + 1), :].rearrange(
                        "e (g c) -> e g c", c=128)[:, :, 32 * i: 32 * i + 32],
                )
            am = chp.tile([KC, DD], F32, tag="am")
            nc.sync.dma_start(am[:], s_dr[:])

            # bf16 x input DMAs: queued on sync after the stat scatters so
            # the transfers run during the factorization window (DMA is
            # idle there); pass-2 transposes pick each chunk up as it lands.
            for ci in range(NCH):
                nc.sync.dma_start(
                    xbt[ci][:].rearrange("p (j c) -> p j c", j=4),
                    xs[:][512 * ci: 512 * (ci + 1), :].rearrange(
                        "(j p) c -> p j c", j=4),
                )

            # ---------------- cov assembly (am = new_cov + I) ----------------
            av = am[:].rearrange("p (e d) -> p e d", d=D)
            xbar = chp.tile([KC, D], F32, tag="xbar")
            nc.vector.tensor_scalar_mul(xbar[:], t_k[:], 1.0 / B)
            xd = chp.tile([KC, D], F32, tag="xd")
            nc.vector.tensor_sub(xd[:], xbar[:], mu0t[:])
            nmu = chp.tile([KC, D], F32, tag="nmu")
            nc.vector.tensor_add(nmu[:], n0mu0[:], t_k[:])
            nc.vector.tensor_scalar_mul(nmu[:], nmu[:], invden[:])
            tmp1 = chp.tile([KC, DD], F32, tag="tmp1")
            tv = tmp1[:].rearrange("p (e d) -> p e d", d=D)
            nc.vector.tensor_tensor(
                tv,
                t_k[:].unsqueeze(2).broadcast_to([KC, D, D]),
                xbar[:].unsqueeze(1).broadcast_to([KC, D, D]),
                ALU.mult,
            )
            nc.vector.tensor_sub(am[:], am[:], tmp1[:])
            nc.vector.scalar_tensor_tensor(
                am[:], am[:], invden[:], ghat[:], ALU.mult, ALU.add)
            nc.vector.tensor_tensor(
                tv,
                xd[:].unsqueeze(2).broadcast_to([KC, D, D]),
                xd[:].unsqueeze(1).broadcast_to([KC, D, D]),
                ALU.mult,
            )
            nc.vector.scalar_tensor_tensor(
                am[:], tmp1[:], coefx[:], am[:], ALU.mult, ALU.add)

            # ---------------- LDL^T factorization (vector engine) ----------
            for j in range(D - 1):
                n = D - 1 - j
                rawc = am[:, 32 * (j + 1) + j: DD: 32]
                invd = chtmp.tile([KC, 1], F32, tag="invd")
                nc.vector.reciprocal(invd[:], am[:, 33 * j: 33 * j + 1])
                nc.vector.tensor_scalar_mul(invd[:], invd[:], -1.0)
                tmpu = chtmp.tile([KC, 31, 31], F32, tag="tmpu")
                nc.vector.tensor_tensor(
                    tmpu[:, 0:n, 0:n],
                    rawc.unsqueeze(2).broadcast_to([KC, n, n]),
                    rawc.unsqueeze(1).broadcast_to([KC, n, n]),
                    ALU.mult,
                )
                nc.vector.scalar_tensor_tensor(
                    av[:, j + 1: D, j + 1: D],
                    tmpu[:, 0:n, 0:n],
                    invd[:],
                    av[:, j + 1: D, j + 1: D],
                    ALU.mult, ALU.add,
                )

            dv = chp.tile([KC, D], F32, tag="dv")
            nc.vector.tensor_copy(dv[:], am[:, 0:DD:33])
            rdv = chp.tile([KC, D], F32, tag="rdv")
            nc.vector.reciprocal(rdv[:], dv[:])
            # unit-lower L: scale columns by 1/d (upper/diag junk unused)
            ltmp = tmp1
            nc.vector.tensor_tensor(
                ltmp[:].rearrange("p (e d) -> p e d", d=D),
                av,
                rdv[:].unsqueeze(1).broadcast_to([KC, D, D]),
                ALU.mult,
            )

            # ------------- pass-2 prep: transpose resident x IN PLACE -------
            # Emitted here so PE/scalar overlap the vector-engine chain.
            # (The scalar sqrt below is intentionally AFTER most copies in
            # the scalar queue: rsq isn't needed until the final row scale.)
            NCH_EARLY = 18   # rest emitted after the inverse loop (fills the
            #                  PE idle gap so it stays warm for the whitens)
            with tc.tile_pool(name="xps", bufs=2, space="PSUM") as xps:

                def emit_xpose(c):
                    pxt = xps.tile([P, 4 * COLS], BF16, tag="pxt")
                    for j in range(4):
                        xt = xbv(4 * c + j)
                        for g in range(4):
                            nc.tensor.transpose(
                                pxt[:, 512 * g + 128 * j:
                                    512 * g + 128 * (j + 1)],
                                xt[:, 128 * g: 128 * (g + 1)],
                                idt[:],
                            )
                    nc.scalar.copy(xbt[c][:], pxt[:])

                for c in range(NCH_EARLY):
                    emit_xpose(c)

                # rsq = 1/sqrt(d): scalar sqrt + 2 Newton steps on vector
                rsq = chp.tile([KC, D], F32, tag="rsq")
                nc.scalar.activation(rsq[:], rdv[:], ACTF.Sqrt)

                # ---------------- unit-lower inverse ----------------
                wv = wu[:].rearrange("p (i c) -> p i c", c=D)
                for jc in range(D - 1):
                    n = D - 1 - jc
                    lcol = ltmp[:, 32 * (jc + 1) + jc: DD: 32]
                    roww = wv[:, jc, 0: jc + 1]
                    tmpu = chtmp.tile([KC, 31, 31], F32, tag="tmpu")
                    nc.vector.tensor_tensor(
                        tmpu[:, 0:n, 0: jc + 1],
                        lcol.unsqueeze(2).broadcast_to([KC, n, jc + 1]),
                        roww.unsqueeze(1).broadcast_to([KC, n, jc + 1]),
                        ALU.mult,
                    )
                    nc.vector.tensor_sub(
                        wv[:, jc + 1: D, 0: jc + 1],
                        wv[:, jc + 1: D, 0: jc + 1],
                        tmpu[:, 0:n, 0: jc + 1],
                    )

                for c in range(NCH_EARLY, NCH):
                    emit_xpose(c)

                nt1 = chp.tile([KC, D], F32, tag="nt1")
                for _ in range(2):
                    nc.vector.tensor_tensor(nt1[:], rsq[:], rsq[:], ALU.mult)
                    nc.vector.tensor_tensor(nt1[:], nt1[:], dv[:], ALU.mult)
                    nc.vector.tensor_scalar(
                        out=nt1[:], in0=nt1[:], scalar1=-0.5, scalar2=1.5,
                        op0=ALU.mult, op1=ALU.add,
                    )
                    nc.vector.tensor_tensor(rsq[:], rsq[:], nt1[:], ALU.mult)

                # scale rows by 1/sqrt(d)
                nc.vector.tensor_tensor(
                    wv, wv,
                    rsq[:].unsqueeze(2).broadcast_to([KC, D, D]), ALU.mult)

                # W^T (e-major) in bf16, scattered to block-diag wblk first
                # (the whitens need wblk; the bias path can lag)
                wt16 = chp.tile([KC, DD], BF16, tag="wt16")
                nc.vector.tensor_copy(
                    wt16[:].rearrange("p (e d) -> p e d", d=D),
                    wv.transpose([0, 2, 1]),
                )
                wt_dr = dr.tile([KC, DD], BF16, tag="wt_dr")
                nc.sync.dma_start(wt_dr[:], wt16[:])
                for i in range(4):
                    nc.sync.dma_start(
                        wblk[32 * i: 32 * (i + 1), :].rearrange(
                            "e (g c) -> e g c", c=128)[
                                :, :, 32 * i: 32 * i + 32],
                        wt_dr[:].rearrange("(g f) c -> f g c", f=4)[i]
                        .rearrange("g (e d) -> e g d", d=D),
                    )

                # bias = -W @ new_mu  (per cluster)
                nc.vector.tensor_tensor(
                    ltmp[:].rearrange("p (d e) -> p d e", e=D),
                    wv,
                    nmu[:].unsqueeze(1).broadcast_to([KC, D, D]),
                    ALU.mult,
                )
                wmu = chp.tile([KC, D], F32, tag="wmu")
                nc.vector.tensor_reduce(
                    wmu[:], ltmp[:].rearrange("p (d e) -> p d e", e=D),
                    mybir.AxisListType.X, ALU.add,
                )
                nc.vector.tensor_scalar_mul(wmu[:], wmu[:], -1.0)
                wm_dr = dr.tile([KC, D], F32, tag="wm_dr")
                nc.sync.dma_start(wm_dr[:], wmu[:])
                # flat(wm_dr)[k'*32+d] = flat[128*g + (32*i+d)] -> [p, g]
                nc.sync.dma_start(
                    bias[:],
                    wm_dr[:].rearrange("(g i) d -> g (i d)", i=4)
                    .transpose([1, 0]),
                )

            # ---------------- pass 2: whiten ----------------
            with (
                tc.tile_pool(name="zps", bufs=7, space="PSUM") as zps,
                tc.tile_pool(name="wrm", bufs=1, space="PSUM") as wrm,
            ):
                # PE p-state warm-up: ~3us of dummy transposes gated on
                # wblk so the whitens start at full clock.
                scr = wrm.tile([1, P], BF16, tag="scr")
                for _ in range(20):
                    nc.tensor.transpose(scr[:], wblk[:, 0:1], idt[:])
                # z staging: per-group [128, 1024] tiles covering chunk
                # pairs -> 64 output DMAs, all on the (otherwise idle) sync
                # queue.  Bias adds rotate scalar/vector/gpsimd.
                zstp = [zb0, zb1, zb2, zb3]
                tg = [None] * 4
                for c in range(NCH):
                    for g in range(4):
                        pz = zps.tile([P, 512], F32, tag="pz")
                        nc.tensor.matmul(
                            pz[:],
                            wblk[:, 128 * g: 128 * (g + 1)],
                            xbt[c][:, 512 * g: 512 * (g + 1)],
                            start=True, stop=True,
                        )
                        if c % 2 == 0:
                            tg[g] = zstp[g].tile([P, 1024], BF16,
                                                 tag=f"zb{g}",
                                                 name=f"zb{g}_{c}")
                        half = tg[g][:, 512 * (c % 2): 512 * (c % 2 + 1)]
                        on_scalar = (g == 0) or (g == 2 and c % 2 == 0) \
                            or (g == 3 and c % 2 == 1)
                        if on_scalar:
                            nc.scalar.activation(
                                half, pz[:], ACTF.Identity,
                                bias=bias[:, g: g + 1])
                        else:
                            nc.vector.tensor_scalar_add(
                                half, pz[:], bias[:, g: g + 1])
                        if c % 2 == 1:
                            nc.sync.dma_start(
                                zt_out[:][128 * g: 128 * (g + 1),
                                          512 * (c - 1): 512 * (c + 1)],
                                tg[g][:],
                            )

    nc.compile()
    return nc


def _get_nc():
    if "nc" not in _CACHE:
        _CACHE["nc"] = _build()
    return _CACHE["nc"]


def kernel(x, mu_0, L_0, n_0):
    x = np.asarray(x, dtype=np.float32)
    mu_0 = np.asarray(mu_0, dtype=np.float32)
    L_0 = np.asarray(L_0, dtype=np.float32)
    n_0 = np.asarray(n_0, dtype=np.float32)

    nc = _get_nc()

    n0 = float(n_0[0])
    denom = n0 + B
    invden = 1.0 / denom
    coefg = n0 / denom
    coefx = n0 * B / (denom * denom)
    scal = np.array([[invden, coefx]], dtype=np.float32)
    idt = np.eye(P, dtype=ml_dtypes.bfloat16)
    ones = np.ones((P, 1), dtype=ml_dtypes.bfloat16)
    fp8 = mybir.dt.np(mybir.dt.float8e4)
    ones8 = np.ones((P, 128), dtype=fp8)
    eye = np.broadcast_to(
        np.eye(D, dtype=np.float32).reshape(1, DD), (KC, DD)).copy()
    mu0t_full = np.ascontiguousarray(mu_0.T)          # [K, D]
    g_full = np.einsum('kde,kfe->kdf', L_0, L_0)      # [K, D, D]

    # per-core slabs: xr2[c] = [B, 512] cluster-major (col = k'*32 + d)
    xr = np.ascontiguousarray(x.transpose(0, 2, 1))   # [B, K, D]
    xr2 = np.ascontiguousarray(
        xr.reshape(B, N_CORES, COLS).transpose(1, 0, 2))  # [8, B, 512]

    in_maps = []
    for c in range(N_CORES):
        sl = slice(KC * c, KC * (c + 1))
        ghat = (g_full[sl].reshape(KC, DD) * coefg
                + eye).astype(np.float32)
        in_maps.append({
            "xs": xr2[c].astype(ml_dtypes.bfloat16),
            "xs8": xr2[c].astype(fp8),
            "ones8_in": ones8,
            "ghat_in": np.ascontiguousarray(ghat),
            "n0mu0_in": np.ascontiguousarray(n0 * mu0t_full[sl]),
            "mu0t_in": np.ascontiguousarray(mu0t_full[sl]),
            "scal_in": scal,
            "eye_in": eye,
            "idt_in": idt,
            "ones_in": ones,
        })
    res = run_bass_kernel_spmd(
        nc, in_maps, core_ids=list(range(N_CORES)),
        trace=bool(_CACHE.get("trace", False)),
    )
    _CACHE["last_res"] = res

    z = np.empty((B, D, K), dtype=np.float32)
    for c in range(N_CORES):
        zt = np.asarray(res.results[c]["zt_out"],
                        dtype=np.float32)            # [512, B]
        # row = 128*g + 32*i + d  ->  cluster k' = 4*g + i, feature d
        zc = zt.reshape(4, 4, D, B).transpose(3, 2, 0, 1).reshape(B, D, KC)
        z[:, :, KC * c: KC * (c + 1)] = zc
    return z
